# revision 34
# baseline (speedup 1.0000x reference)
"""Trainium2 Bass kernel for nn_NodeNet (GNN message passing + 15-qubit circuit).

Exact algebraic structure exploited:
1. The joint state stays a tensor product of small components; only <Z_5>,
   <Z_10> are measured, and the measurement factorizes through the product.
   Largest live state tile: 16 floats per node.
2. Every angle in the circuit is affine in (X cols, mi/mo cols, theta): all
   theta sums are folded into one [32 x 58] host constant matrix CK.
   A single PE matmul (theta-replicated [32,128] lhsT x CK) produces the
   per-node fold rows frep [128, 58]; no on-device theta add-trees.
3. All sincos pairs come from magic-round + one Sin per group: the cos rows
   carry a +0.25-turn shift on CK's ones-row, since the device wn is
   round(s)-s and sin(2pi*rr(s+1/4)) = cos(2pi*s).
   Group A (pack-gated): X-block pairs + cs pairs -> vA [128, 26].
   Group B (ANGM-gated): the 8 M-dependent quads -> v2L [128, 32].
4. The q10/z10 branch (chain B) runs on Pool right after group A, well
   before the z5 spine finishes.

Dtypes: Ri/Ro move as fp8 (one-hot entries exact); measured end-to-end rel
err ~3e-3 vs the 2e-2 harness gate.

Inputs packed into 3 DMAs (RiRo, pack, RT): RiRo fp8 [128,2048], RT fp8
(chunked transposes), PACK [128,199] f32 = X | e-transposed | CK [32x58] |
theta-replicated [32x128]. Self-contained.
"""

import math

import numpy as np

N_CORES = 8
PI = math.pi
MAGIC = 12582912.0          # 1.5 * 2^23: f32 round-to-nearest-integer bias
K4 = 1.0 / (4.0 * PI)       # full angle -> turns of half-angle

# pack columns
PX = 0            # X[:, 0:5]
PE0 = 5           # e transposed: pack[p, 5+c] = e[c*128+p]
PCK = 13          # CK fold matrix [32, 58] (rows 0:32)
NCK = 58
PTHR = PCK + NCK  # theta replicated [32, 128] (rows 0:32), row31 = 1
PW = PTHR + 128   # 199

# cs-pair indices within vA (base 10 + 4*(i//2))
CS25, CS19, CSB, CS29, CS30, CSPA, CSNA = range(7)


def build_ck():
    """theta coefficients of the 16 late fold columns (K4-scaled),
    pair-major [+, -] per angle quad A0..A7."""
    ck = np.zeros((31, 16), dtype=np.float64)
    shifts = [None, 15, 16, None, None, 14, 15, None]
    for q in range(8):
        cp, cm = 2 * q, 2 * q + 1
        ck[q, cp] += K4
        ck[q, cm] += K4
        if shifts[q] is not None:
            ck[shifts[q], cp] += K4
            ck[shifts[q], cm] -= K4
    return ck


def build_ck58():
    """CK [36, 58]: rows 0:31 theta, row 31 ones (+0.25 per cos column),
    rows 32:36 the X columns (X1, X3, X4, X0) folded into the matmul.

    cols 0:32   late quads 0..7, layout [c+, s+, s-, c-] per quad.
    cols 32:42  X-block pairs j=0..4 as [c, s]:
                j=0 q11a(X1), j=1 q11b(X3), j=2 M14(2*X4),
                j=3 q10+(X0), j=4 q10-(X0).
    cols 42:58  cs quads: quad k holds cs pairs (2k, 2k+1) as
                [c_a, s_a, s_b, c_b]; cs7 is a zero dummy.
    """
    ck16 = build_ck()
    ck = np.zeros((36, NCK), dtype=np.float64)
    for q in range(8):
        ck[0:31, 4 * q + 0] = ck16[:, 2 * q]
        ck[0:31, 4 * q + 1] = ck16[:, 2 * q]
        ck[0:31, 4 * q + 2] = ck16[:, 2 * q + 1]
        ck[0:31, 4 * q + 3] = ck16[:, 2 * q + 1]
        ck[31, 4 * q + 0] = 0.25
        ck[31, 4 * q + 3] = 0.25
    # X-block: (theta combo, X row, X coeff)
    xb = [([(11, 1.0)], 32, 1.0),
          ([(13, 1.0), (18, 1.0), (22, 1.0)], 33, 1.0),
          ([(14, 2.0), (19, 2.0), (28, 2.0)], 34, 2.0),
          ([(10, 1.0), (17, 1.0), (21, 1.0)], 35, 1.0),
          ([(10, 1.0), (17, -1.0), (21, -1.0)], 35, 1.0)]
    for j, (combo, xrow, xw) in enumerate(xb):
        cc, sc = 32 + 2 * j, 33 + 2 * j
        for r, w in combo:
            ck[r, cc] += K4 * w
            ck[r, sc] += K4 * w
        ck[xrow, cc] += K4 * xw
        ck[xrow, sc] += K4 * xw
        ck[31, cc] += 0.25
    # cs combos
    cs = [[(25, 1.0)], [(19, 1.0)], [(24, 1.0), (27, 1.0)],
          [(29, 2.0)], [(30, 2.0)],
          [(20, 1.0), (23, 1.0), (26, 1.0)],
          [(20, 1.0), (23, -1.0), (26, -1.0)], []]
    for i, combo in enumerate(cs):
        qc = 42 + 4 * (i // 2)
        ca, sa = (qc + 0, qc + 1) if i % 2 == 0 else (qc + 3, qc + 2)
        for r, w in combo:
            ck[r, ca] += K4 * w
            ck[r, sa] += K4 * w
        ck[31, ca] += 0.25
    return ck.astype(np.float32)


_cache = {}


def _build_program():
    import concourse.bacc as bacc
    import concourse.mybir as mybir
    import concourse.tile as tile

    f32 = mybir.dt.float32
    bf16 = mybir.dt.bfloat16
    fp8 = mybir.dt.float8e4
    Alu = mybir.AluOpType
    Act = mybir.ActivationFunctionType
    Ax = mybir.AxisListType

    nc = bacc.Bacc(
        "TRN2",
        target_bir_lowering=False,
        debug=False,
        enable_asserts=False,
        num_devices=1,
    )

    RiRo_d = nc.dram_tensor("RiRo_f8", [128, 2048], fp8,
                            kind="ExternalInput").ap()
    RT_d = nc.dram_tensor("RT_f8", [128, 2048], fp8,
                          kind="ExternalInput").ap()
    pk_d = nc.dram_tensor("pack", [128, PW], f32, kind="ExternalInput").ap()
    out_d = nc.dram_tensor("out", [128, 2], f32, kind="ExternalOutput").ap()

    with tile.TileContext(nc) as tc:
        with (
            tc.tile_pool(name="sbuf", bufs=1) as sb,
            tc.tile_pool(name="psum", bufs=1, space="PSUM") as ps,
        ):
            # ---------------- input DMAs ----------------
            RiRo = sb.tile([128, 2048], fp8, tag="RiRo")
            RT = sb.tile([128, 2048], fp8, tag="RT")
            pk = sb.tile([128, PW], f32, tag="pack")
            nc.gpsimd.dma_start(pk[:], pk_d)
            nc.sync.dma_start(RiRo[:], RiRo_d)
            nc.sync.dma_start(RT[:], RT_d)

            # ---------------- constants ----------------
            pit = sb.tile([128, 1], f32, tag="pit")
            nc.gpsimd.memset(pit[:], PI)
            warm = sb.tile([128, 1], f32, tag="warm")
            nc.gpsimd.memset(warm[:], 0.0)
            nc.scalar.activation(warm[:], warm[:], Act.Sin)

            # ---------------- tiles ----------------
            frep = ps.tile([128, NCK], f32, tag="frep")
            frepB = sb.tile([128, 32], f32, tag="frepB")
            bb_ps = ps.tile([128, 64], f32, tag="bb")
            ANGM = ps.tile([128, 8], f32, tag="ANGM")
            X_bf = sb.tile([128, 5], bf16, tag="Xbf")
            bw = sb.tile([128, 64], bf16, tag="bw")

            # X cast first: PE stage-1 reads it
            nc.vector.tensor_copy(X_bf[:], pk[:, PX:PX + 5])

            # ============ PE: fold matmul + stage-1 ============
            nc.tensor.matmul(frep[:], pk[0:36, PTHR:PTHR + 128],
                             pk[0:36, PCK:PCK + NCK], start=True, stop=True)
            nc.scalar.activation(frepB[:], frep[:, 0:32], Act.Identity)
            for c in range(8):
                nc.tensor.matmul(bb_ps[:, c * 8 + 5:c * 8 + 8],
                                 RiRo[:, c * 128:(c + 1) * 128],
                                 X_bf[:, 0:3], start=True, stop=True)
            for c in range(8):
                nc.tensor.matmul(bb_ps[:, c * 8:c * 8 + 5],
                                 RiRo[:, 1024 + c * 128:1024 + (c + 1) * 128],
                                 X_bf[:], start=True, stop=True)

            # ============ group A trig: X-block + cs (pure frep cols) ====
            tA = sb.tile([128, 26], f32, tag="tA")
            wnA = sb.tile([128, 26], f32, tag="wnA")
            vA = sb.tile([128, 26], f32, tag="vA")

            nc.vector.tensor_scalar(tA[:], frep[:, 32:58], MAGIC, None,
                                    Alu.add)
            nc.vector.scalar_tensor_tensor(
                wnA[:], tA[:], MAGIC, frep[:, 32:58], Alu.subtract,
                Alu.subtract)
            nc.scalar.activation(vA[:], wnA[:], Act.Sin, scale=-2.0 * PI)

            def cs_c(i):
                b = 10 + 4 * (i // 2)
                return vA[:, b:b + 1] if i % 2 == 0 else vA[:, b + 3:b + 4]

            def cs_s(i):
                b = 10 + 4 * (i // 2)
                return vA[:, b + 1:b + 2] if i % 2 == 0 else vA[:, b + 2:b + 3]

            def cs_pair(i):
                b = 10 + 4 * (i // 2)
                if i % 2 == 0:
                    return vA[:, b:b + 2]
                return vA[:, b + 3:b + 1:-1]

            # ============ weights + stage-2 ============
            ev = pk[:, PE0:PE0 + 8].rearrange("p (c o) -> p c o", o=1)
            nc.vector.tensor_tensor(
                bw[:].rearrange("p (c j) -> p c j", j=8),
                bb_ps[:].rearrange("p (c j) -> p c j", j=8),
                ev.to_broadcast((128, 8, 8)), Alu.mult)
            for c in range(8):
                nc.tensor.matmul(ANGM[:, 0:5],
                                 RT[:, c * 128:(c + 1) * 128],
                                 bw[:, c * 8:c * 8 + 5],
                                 start=(c == 0), stop=(c == 7))
            for c in range(8):
                nc.tensor.matmul(ANGM[:, 5:8],
                                 RT[:, 1024 + c * 128:1024 + (c + 1) * 128],
                                 bw[:, c * 8 + 5:c * 8 + 8],
                                 start=(c == 0), stop=(c == 7))

            # ============ group B trig: the 8 M-quads ============
            s1L = sb.tile([128, 32], f32, tag="s1L")
            t1L = sb.tile([128, 32], f32, tag="t1L")
            wnL = sb.tile([128, 32], f32, tag="wnL")
            v2L = sb.tile([128, 32], f32, tag="v2L")

            nc.vector.scalar_tensor_tensor(
                s1L[:].rearrange("p (q d) -> p q d", d=4),
                ANGM[:].unsqueeze(2).to_broadcast((128, 8, 4)), K4,
                frepB[:].rearrange("p (q d) -> p q d", d=4),
                Alu.mult, Alu.add)
            nc.vector.tensor_scalar(t1L[:], s1L[:], MAGIC, None, Alu.add)
            nc.vector.scalar_tensor_tensor(
                wnL[:], t1L[:], MAGIC, s1L[:], Alu.subtract, Alu.subtract)
            nc.scalar.activation(v2L[:], wnL[:], Act.Sin, scale=-2.0 * PI)

            v2qv = v2L[:].rearrange("p (q f) -> p q f", f=4)

            # ---- chain B tiles (ops emitted last) ----
            m4 = sb.tile([128, 4], f32, tag="m4")
            m8 = sb.tile([128, 8], f32, tag="m8")
            edB = sb.tile([128, 16], f32, tag="edB")
            zb = sb.tile([128, 8], f32, tag="zb")
            zA = sb.tile([128, 4], f32, tag="zA")
            scr2 = sb.tile([128, 8], f32, tag="scr2")

            # ============ chain A spine (DVE) ============
            m01 = sb.tile([128, 8], f32, tag="m01")
            m5 = sb.tile([128, 16], f32, tag="m5")
            D16 = sb.tile([128, 16], f32, tag="D16")

            m01r = m01[:].rearrange("p (k a b) -> p k a b", a=2, b=2)
            nc.vector.tensor_tensor(
                m01r,
                v2qv[:, 0:4:3, 0:2].unsqueeze(2).to_broadcast((128, 2, 2, 2)),
                v2qv[:, 1:3, :].rearrange("p k (b a) -> p k a b", a=2, b=2),
                Alu.mult)

            def dve_mc(dst, L, H, da, db, bc, bt, S):
                v1, v0 = da >> (bc + 1), 1 << bc
                tbh = bt - S
                w1, w0 = db >> (tbh + 1), 1 << tbh
                ov = dst.rearrange(
                    "p (w1 tb w0 v1 cb v0) -> p w1 tb w0 v1 cb v0",
                    tb=2, cb=2, w0=w0, v0=v0, w1=w1, v1=v1)
                Lv = L.rearrange("p (v1 cb v0) -> p v1 cb v0", cb=2, v0=v0)
                Hv = H.rearrange("p (w1 tb w0) -> p w1 tb w0", tb=2, w0=w0)
                for cbit in range(2):
                    o_h = ov[:, :, :, :, :, cbit, :]
                    Lh = Lv[:, :, cbit, :].unsqueeze(1).unsqueeze(1).unsqueeze(1)
                    Hh = Hv if cbit == 0 else Hv[:, :, ::-1, :]
                    Hh = Hh.unsqueeze(4).unsqueeze(5)
                    nc.vector.tensor_tensor(
                        o_h.squeeze(),
                        Lh.to_broadcast((128, w1, 2, w0, v1, v0)).squeeze(),
                        Hh.to_broadcast((128, w1, 2, w0, v1, v0)).squeeze(),
                        Alu.mult)

            dve_mc(m5[:], m01[:, 0:4], m01[:, 4:8], 4, 4, 1, 3, 2)

            def dve_ry(v, D, b, c_ap, s_ap, F, dst=None):
                if dst is None:
                    dst = v
                nc.vector.tensor_scalar(D[:, 0:F], v, s_ap, None, Alu.mult)
                vv = v.rearrange("p (o t i) -> p o t i", t=2, i=1 << b)
                dv = dst.rearrange("p (o t i) -> p o t i", t=2, i=1 << b)
                Dv = D[:, 0:F].rearrange("p (o t i) -> p o t i", t=2,
                                         i=1 << b)
                nc.vector.scalar_tensor_tensor(
                    dv[:, :, 0], vv[:, :, 0], c_ap, Dv[:, :, 1],
                    Alu.mult, Alu.subtract)
                nc.vector.scalar_tensor_tensor(
                    dv[:, :, 1], vv[:, :, 1], c_ap, Dv[:, :, 0],
                    Alu.mult, Alu.add)

            dve_ry(m5[:], D16, 0, cs_c(CS25), cs_s(CS25), 16)
            dve_ry(m5[:], D16, 3, cs_c(CS19), cs_s(CS19), 16)

            # ============ m6/pn side (Pool) ============
            def pool_mc(dst, L, H, da, db, bc, bt, S):
                v1, v0 = da >> (bc + 1), 1 << bc
                tbh = bt - S
                w1, w0 = db >> (tbh + 1), 1 << tbh
                ov = dst.rearrange(
                    "p (w1 tb w0 v1 cb v0) -> p w1 tb w0 v1 cb v0",
                    tb=2, cb=2, w0=w0, v0=v0, w1=w1, v1=v1)
                Lv = L.rearrange("p (v1 cb v0) -> p v1 cb v0", cb=2, v0=v0)
                Hv = H.rearrange("p (w1 tb w0) -> p w1 tb w0", tb=2, w0=w0)
                for cbit in range(2):
                    o_h = ov[:, :, :, :, :, cbit, :]
                    Lh = Lv[:, :, cbit, :].unsqueeze(1).unsqueeze(1).unsqueeze(1)
                    Hh = Hv if cbit == 0 else Hv[:, :, ::-1, :]
                    Hh = Hh.unsqueeze(4).unsqueeze(5)
                    nc.gpsimd.tensor_tensor(
                        o_h.squeeze(),
                        Lh.to_broadcast((128, w1, 2, w0, v1, v0)).squeeze(),
                        Hh.to_broadcast((128, w1, 2, w0, v1, v0)).squeeze(),
                        Alu.mult)

            m32 = sb.tile([128, 8], f32, tag="m32")
            m6 = sb.tile([128, 16], f32, tag="m6")
            pn = sb.tile([128, 32], f32, tag="pn")
            edA = sb.tile([128, 64], f32, tag="edA")
            edN = sb.tile([128, 64], f32, tag="edN")

            m32r = m32[:].rearrange("p (k a b) -> p k a b", a=2, b=2)
            nc.gpsimd.tensor_tensor(
                m32r,
                v2qv[:, 7:3:-3, 0:2].unsqueeze(2).to_broadcast((128, 2, 2, 2)),
                v2qv[:, 6:4:-1, :].rearrange("p k (b a) -> p k a b", a=2, b=2),
                Alu.mult)
            pool_mc(m6[:], m32[:, 0:4], m32[:, 4:8], 4, 4, 1, 3, 2)

            def pool_ry_ed(v, ed, pair_ap, b, F, dst):
                edv = ed[:, 0:2 * F].rearrange("p (t f) -> p t f", t=2)
                nc.gpsimd.tensor_tensor(
                    edv, v.unsqueeze(1).to_broadcast((128, 2, F)),
                    pair_ap.unsqueeze(2).to_broadcast((128, 2, F)), Alu.mult)
                E = ed[:, 0:F]
                D = ed[:, F:2 * F]
                i = 1 << b
                vv = dst.rearrange("p (o t i) -> p o t i", t=2, i=i)
                Ev = E.rearrange("p (o t i) -> p o t i", t=2, i=i)
                Dv = D.rearrange("p (o t i) -> p o t i", t=2, i=i)
                nc.gpsimd.tensor_tensor(vv[:, :, 0], Ev[:, :, 0], Dv[:, :, 1],
                                        Alu.subtract)
                nc.gpsimd.tensor_tensor(vv[:, :, 1], Ev[:, :, 1], Dv[:, :, 0],
                                        Alu.add)

            pool_ry_ed(m6[:], edA, cs_pair(CSPA), 3, 16, dst=pn[:, 0:16])
            dve_ry(m6[:], edN, 3, cs_c(CSNA), cs_s(CSNA), 16,
                   dst=pn[:, 16:32])

            # ============ measurement sums (DVE tail) ============
            zacc = sb.tile([128, 8], f32, tag="zacc")
            sq5 = sb.tile([128, 16], f32, tag="sq5")
            dD = sb.tile([128, 8], f32, tag="dD")
            scr3 = sb.tile([128, 32], f32, tag="scr3")
            nc.vector.tensor_tensor(sq5[:], m5[:], m5[:], Alu.mult)
            nc.vector.tensor_tensor(dD[:, 0:4], sq5[:, 0:8:2], sq5[:, 1:8:2],
                                    Alu.subtract)
            nc.vector.tensor_tensor(dD[:, 4:8], sq5[:, 9:16:2],
                                    sq5[:, 8:16:2], Alu.subtract)
            nc.vector.tensor_reduce(
                zacc[:, 0:2],
                dD[:].rearrange("p (a b) -> p a b", b=4), Ax.X, Alu.add)
            nc.vector.tensor_reduce(
                zacc[:, 2:4],
                sq5[:].rearrange("p (a b) -> p a b", b=8), Ax.X, Alu.add)
            nc.vector.scalar_tensor_tensor(
                scr3[:, 0:8], pn[:, 0:8], 2.0, pn[:, 0:8], Alu.mult, Alu.mult,
                accum_out=zacc[:, 4:5])
            nc.vector.scalar_tensor_tensor(
                scr3[:, 8:16], pn[:, 16:24], 2.0, pn[:, 16:24], Alu.mult,
                Alu.mult, accum_out=zacc[:, 5:6])
            nc.vector.scalar_tensor_tensor(
                scr3[:, 16:24], pn[:, 0:8], -2.0, pn[:, 8:16], Alu.mult,
                Alu.mult, accum_out=zacc[:, 6:7])
            nc.vector.scalar_tensor_tensor(
                scr3[:, 24:32], pn[:, 16:24], -2.0, pn[:, 24:32], Alu.mult,
                Alu.mult, accum_out=zacc[:, 7:8])

            # ============ final assembly ============
            out_sb = sb.tile([128, 2], f32, tag="out")
            cM14 = vA[:, 4:5]

            # z5: pairing [Sm, -Tm, Sp, Tp] x [2A0p, 2A0n, -2Qp, -2Qn]
            G = sb.tile([128, 4], f32, tag="G")
            zf = sb.tile([128, 4], f32, tag="zf")
            nc.vector.tensor_tensor(
                G[:].rearrange("p (a b) -> p a b", b=2),
                zacc[:, 4:8].rearrange("p (a b) -> p a b", b=2),
                cs_pair(CS29).unsqueeze(2).to_broadcast((128, 2, 2)),
                Alu.mult)
            nc.vector.scalar_tensor_tensor(
                zf[:, 0:4], G[:], 1.0, zacc[:, 0:4], Alu.mult, Alu.mult,
                accum_out=zf[:, 0:1])
            nc.scalar.activation(zf[:, 1:2], zacc[:, 0:1], Act.Identity,
                                 scale=1.0, bias=zacc[:, 1:2])
            nc.scalar.activation(zf[:, 2:3], zf[:, 1:2], Act.Identity,
                                 scale=cs_c(CS29))
            nc.scalar.activation(zf[:, 3:4], zf[:, 2:3], Act.Identity,
                                 scale=-1.0, bias=zf[:, 0:1])
            nc.scalar.activation(out_sb[:, 0:1], zf[:, 3:4], Act.Identity,
                                 scale=-PI, bias=pit[:])

            # ============ chain B (q10/z10): lowest priority ====

            nc.gpsimd.tensor_tensor(m4[:, 0:2], vA[:, 0:2], vA[:, 6:10:3],
                                    Alu.mult)
            nc.gpsimd.tensor_tensor(m4[:, 2:4], vA[:, 0:2], vA[:, 7:9],
                                    Alu.mult)

            pool_mc(m8[:], vA[:, 2:4], m4[:], 2, 4, 0, 2, 1)

            edv8 = edB[:, 0:16].rearrange("p (t f) -> p t f", t=2)
            nc.gpsimd.tensor_tensor(
                edv8, m8[:].unsqueeze(1).to_broadcast((128, 2, 8)),
                cs_pair(CSB).unsqueeze(2).to_broadcast((128, 2, 8)), Alu.mult)
            m8v = m8[:].rearrange("p (o t i) -> p o t i", t=2, i=4)
            E8 = edB[:, 0:8].rearrange("p (o t i) -> p o t i", t=2, i=4)
            D8 = edB[:, 8:16].rearrange("p (o t i) -> p o t i", t=2, i=4)
            nc.gpsimd.tensor_tensor(m8v[:, :, 0], E8[:, :, 0], D8[:, :, 1],
                                    Alu.subtract)
            nc.gpsimd.tensor_tensor(m8v[:, :, 1], E8[:, :, 1], D8[:, :, 0],
                                    Alu.add)

            # z10 affine coefficients (vA-gated):
            # A1 = -pi*c30*cM14, A2 = pi*s30, A3 = pi*(1 + c30*cM14)
            nc.vector.tensor_tensor(zA[:, 3:4], cM14, vA[:, 18:19], Alu.mult)
            nc.vector.scalar_tensor_tensor(
                zA[:, 2:3], zA[:, 3:4], PI, pit[:], Alu.mult, Alu.add)
            nc.vector.tensor_scalar(zA[:, 0:1], zA[:, 3:4], -PI, None,
                                    Alu.mult)
            nc.vector.tensor_scalar(zA[:, 1:2], vA[:, 19:20], PI, None,
                                    Alu.mult)
            nc.vector.scalar_tensor_tensor(
                scr2[:, 0:4], m8[:, 0:4], 2.0, m8[:, 4:8], Alu.mult, Alu.mult,
                accum_out=zb[:, 1:2])
            nc.vector.scalar_tensor_tensor(
                scr2[:, 4:8], m8[:, 0:4], 2.0, m8[:, 0:4], Alu.mult, Alu.mult,
                accum_out=zb[:, 0:1])
            # z10 late part: out1 = A1*zb0 + A2*zb1 + A3 (A's precomputed)
            nc.vector.scalar_tensor_tensor(
                zb[:, 6:7], zb[:, 0:1], zA[:, 0:1], zA[:, 2:3],
                Alu.mult, Alu.add)
            nc.vector.scalar_tensor_tensor(
                out_sb[:, 1:2], zb[:, 1:2], zA[:, 1:2], zb[:, 6:7],
                Alu.mult, Alu.add)

            nc.sync.dma_start(out_d, out_sb[:])


    nc.compile()
    return nc


def get_nc():
    if "nc" not in _cache:
        _cache["nc"] = _build_program()
    return _cache["nc"]


def kernel(X, e, Ri, Ro, theta):
    import ml_dtypes
    from concourse.bass_utils import run_bass_kernel_spmd

    nc = get_nc()
    X = np.asarray(X, dtype=np.float32)
    e = np.asarray(e, dtype=np.float32)
    theta = np.asarray(theta, dtype=np.float32)
    pack = np.zeros((128, PW), dtype=np.float32)
    pack[:, PX:PX + 5] = X
    pack[:, PE0:PE0 + 8] = e.reshape(8, 128).T
    pack[0:36, PCK:PCK + NCK] = build_ck58()
    pack[0:31, PTHR:PTHR + 128] = np.broadcast_to(theta[:, None], (31, 128))
    pack[31, PTHR:PTHR + 128] = 1.0
    pack[32:36, PTHR:PTHR + 128] = X[:, [1, 3, 4, 0]].T
    f8 = ml_dtypes.float8_e4m3fn

    def chunkT(m):
        return m.T.reshape(8, 128, 128).transpose(1, 0, 2).reshape(128, 1024)

    ri = np.asarray(Ri, dtype=np.float32)
    ro = np.asarray(Ro, dtype=np.float32)
    riro = np.concatenate([ri, ro], axis=1).astype(f8)
    rt = np.concatenate([chunkT(ri), chunkT(ro)], axis=1).astype(f8)
    in_map = {
        "RiRo_f8": np.ascontiguousarray(riro),
        "RT_f8": np.ascontiguousarray(rt),
        "pack": pack,
    }
    res = run_bass_kernel_spmd(
        nc, [dict(in_map) for _ in range(N_CORES)],
        core_ids=list(range(N_CORES)),
    )
    return res.results[0]["out"]


# revision 35
# speedup vs baseline: 1.0301x; 1.0301x over previous
"""Trainium2 Bass kernel for nn_NodeNet (GNN message passing + 15-qubit circuit).

Exact algebraic structure exploited:
1. The joint state stays a tensor product of small components; only <Z_5>,
   <Z_10> are measured, and the measurement factorizes through the product.
   Largest live state tile: 16 floats per node.
2. Every angle in the circuit is affine in (X cols, mi/mo cols, theta): all
   theta sums are folded into one [32 x 58] host constant matrix CK.
   A single PE matmul (theta-replicated [32,128] lhsT x CK) produces the
   per-node fold rows frep [128, 58]; no on-device theta add-trees.
3. All sincos pairs come from magic-round + one Sin per group: the cos rows
   carry a +0.25-turn shift on CK's ones-row, since the device wn is
   round(s)-s and sin(2pi*rr(s+1/4)) = cos(2pi*s).
   Group A (pack-gated): X-block pairs + cs pairs -> vA [128, 26].
   Group B (ANGM-gated): the 8 M-dependent quads -> v2L [128, 32].
4. The q10/z10 branch (chain B) runs on Pool right after group A, well
   before the z5 spine finishes.

Dtypes: Ri/Ro move as fp8 (one-hot entries exact); measured end-to-end rel
err ~3e-3 vs the 2e-2 harness gate.

Inputs packed into 3 DMAs (RiRo, pack, RT): RiRo fp8 [128,2048], RT fp8
(chunked transposes), PACK [128,199] f32 = X | e-transposed | CK [32x58] |
theta-replicated [32x128]. Self-contained.
"""

import math

import numpy as np

N_CORES = 8
PI = math.pi
MAGIC = 12582912.0          # 1.5 * 2^23: f32 round-to-nearest-integer bias
K4 = 1.0 / (4.0 * PI)       # full angle -> turns of half-angle

# pack columns
PX = 0            # X[:, 0:5]
PE0 = 5           # e transposed: pack[p, 5+c] = e[c*128+p]
PCK = 13          # CK fold matrix [32, 58] (rows 0:32)
NCK = 58
PTHR = PCK + NCK  # theta replicated [32, 128] (rows 0:32), row31 = 1
PW = PTHR + 128   # 199

# cs-pair indices within vA (base 10 + 4*(i//2))
CS25, CS19, CSB, CS29, CS30, CSPA, CSNA = range(7)


def build_ck():
    """theta coefficients of the 16 late fold columns (K4-scaled),
    pair-major [+, -] per angle quad A0..A7."""
    ck = np.zeros((31, 16), dtype=np.float64)
    shifts = [None, 15, 16, None, None, 14, 15, None]
    for q in range(8):
        cp, cm = 2 * q, 2 * q + 1
        ck[q, cp] += K4
        ck[q, cm] += K4
        if shifts[q] is not None:
            ck[shifts[q], cp] += K4
            ck[shifts[q], cm] -= K4
    return ck


def build_ck58():
    """CK [36, 58]: rows 0:31 theta, row 31 ones (+0.25 per cos column),
    rows 32:36 the X columns (X1, X3, X4, X0) folded into the matmul.

    cols 0:32   late quads 0..7, layout [c+, s+, s-, c-] per quad.
    cols 32:42  X-block pairs j=0..4 as [c, s]:
                j=0 q11a(X1), j=1 q11b(X3), j=2 M14(2*X4),
                j=3 q10+(X0), j=4 q10-(X0).
    cols 42:58  cs quads: quad k holds cs pairs (2k, 2k+1) as
                [c_a, s_a, s_b, c_b]; cs7 is a zero dummy.
    """
    ck16 = build_ck()
    ck = np.zeros((36, NCK), dtype=np.float64)
    for q in range(8):
        ck[0:31, 4 * q + 0] = ck16[:, 2 * q]
        ck[0:31, 4 * q + 1] = ck16[:, 2 * q]
        ck[0:31, 4 * q + 2] = ck16[:, 2 * q + 1]
        ck[0:31, 4 * q + 3] = ck16[:, 2 * q + 1]
        ck[31, 4 * q + 0] = 0.25
        ck[31, 4 * q + 3] = 0.25
    # X-block: (theta combo, X row, X coeff)
    xb = [([(11, 1.0)], 32, 1.0),
          ([(13, 1.0), (18, 1.0), (22, 1.0)], 33, 1.0),
          ([(14, 2.0), (19, 2.0), (28, 2.0)], 34, 2.0),
          ([(10, 1.0), (17, 1.0), (21, 1.0)], 35, 1.0),
          ([(10, 1.0), (17, -1.0), (21, -1.0)], 35, 1.0)]
    for j, (combo, xrow, xw) in enumerate(xb):
        cc, sc = 32 + 2 * j, 33 + 2 * j
        for r, w in combo:
            ck[r, cc] += K4 * w
            ck[r, sc] += K4 * w
        ck[xrow, cc] += K4 * xw
        ck[xrow, sc] += K4 * xw
        ck[31, cc] += 0.25
    # cs combos
    cs = [[(25, 1.0)], [(19, 1.0)], [(24, 1.0), (27, 1.0)],
          [(29, 2.0)], [(30, 2.0)],
          [(20, 1.0), (23, 1.0), (26, 1.0)],
          [(20, 1.0), (23, -1.0), (26, -1.0)], []]
    for i, combo in enumerate(cs):
        qc = 42 + 4 * (i // 2)
        ca, sa = (qc + 0, qc + 1) if i % 2 == 0 else (qc + 3, qc + 2)
        for r, w in combo:
            ck[r, ca] += K4 * w
            ck[r, sa] += K4 * w
        ck[31, ca] += 0.25
    return ck.astype(np.float32)


_cache = {}


def _build_program():
    import concourse.bacc as bacc
    import concourse.mybir as mybir
    import concourse.tile as tile

    f32 = mybir.dt.float32
    bf16 = mybir.dt.bfloat16
    fp8 = mybir.dt.float8e4
    Alu = mybir.AluOpType
    Act = mybir.ActivationFunctionType
    Ax = mybir.AxisListType

    nc = bacc.Bacc(
        "TRN2",
        target_bir_lowering=False,
        debug=False,
        enable_asserts=False,
        num_devices=1,
    )

    RiRo_d = nc.dram_tensor("RiRo_f8", [128, 2048], fp8,
                            kind="ExternalInput").ap()
    RT_d = nc.dram_tensor("RT_f8", [128, 2048], fp8,
                          kind="ExternalInput").ap()
    pk_d = nc.dram_tensor("pack", [128, PW], f32, kind="ExternalInput").ap()
    out_d = nc.dram_tensor("out", [128, 2], f32, kind="ExternalOutput").ap()

    with tile.TileContext(nc) as tc:
        with (
            tc.tile_pool(name="sbuf", bufs=1) as sb,
            tc.tile_pool(name="psum", bufs=1, space="PSUM") as ps,
        ):
            # ---------------- input DMAs ----------------
            RiRo = sb.tile([128, 2048], fp8, tag="RiRo")
            RT = sb.tile([128, 2048], fp8, tag="RT")
            pk = sb.tile([128, PW], f32, tag="pack")
            nc.gpsimd.dma_start(pk[:], pk_d)
            nc.sync.dma_start(RiRo[:], RiRo_d)
            nc.sync.dma_start(RT[:], RT_d)

            # ---------------- constants ----------------
            pit = sb.tile([128, 1], f32, tag="pit")
            nc.gpsimd.memset(pit[:], PI)
            warm = sb.tile([128, 1], f32, tag="warm")
            nc.gpsimd.memset(warm[:], 0.0)
            nc.scalar.activation(warm[:], warm[:], Act.Sin)

            # ---------------- tiles ----------------
            frep = ps.tile([128, NCK], f32, tag="frep")
            frepB = sb.tile([128, 32], f32, tag="frepB")
            bb_ps = ps.tile([128, 64], f32, tag="bb")
            ANGM = ps.tile([128, 8], f32, tag="ANGM")
            X_bf = sb.tile([128, 5], bf16, tag="Xbf")
            bw = sb.tile([128, 64], bf16, tag="bw")

            # X cast first: PE stage-1 reads it
            nc.vector.tensor_copy(X_bf[:], pk[:, PX:PX + 5])

            # ============ PE: fold matmul + stage-1 ============
            nc.tensor.matmul(frep[:], pk[0:36, PTHR:PTHR + 128],
                             pk[0:36, PCK:PCK + NCK], start=True, stop=True)
            nc.scalar.activation(frepB[:], frep[:, 0:32], Act.Identity)
            for c in range(8):
                nc.tensor.matmul(bb_ps[:, c * 8 + 5:c * 8 + 8],
                                 RiRo[:, c * 128:(c + 1) * 128],
                                 X_bf[:, 0:3], start=True, stop=True)
            for c in range(8):
                nc.tensor.matmul(bb_ps[:, c * 8:c * 8 + 5],
                                 RiRo[:, 1024 + c * 128:1024 + (c + 1) * 128],
                                 X_bf[:], start=True, stop=True)

            # ============ group A trig: X-block + cs (pure frep cols) ====
            tA = sb.tile([128, 26], f32, tag="tA")
            wnA = sb.tile([128, 26], f32, tag="wnA")
            vA = sb.tile([128, 26], f32, tag="vA")

            nc.vector.tensor_scalar(tA[:], frep[:, 32:58], MAGIC, None,
                                    Alu.add)
            nc.vector.scalar_tensor_tensor(
                wnA[:], tA[:], MAGIC, frep[:, 32:58], Alu.subtract,
                Alu.subtract)
            nc.scalar.activation(vA[:], wnA[:], Act.Sin, scale=-2.0 * PI)

            def cs_c(i):
                b = 10 + 4 * (i // 2)
                return vA[:, b:b + 1] if i % 2 == 0 else vA[:, b + 3:b + 4]

            def cs_s(i):
                b = 10 + 4 * (i // 2)
                return vA[:, b + 1:b + 2] if i % 2 == 0 else vA[:, b + 2:b + 3]

            def cs_pair(i):
                b = 10 + 4 * (i // 2)
                if i % 2 == 0:
                    return vA[:, b:b + 2]
                return vA[:, b + 3:b + 1:-1]

            # ============ weights + stage-2 ============
            ev = pk[:, PE0:PE0 + 8].rearrange("p (c o) -> p c o", o=1)
            nc.vector.tensor_tensor(
                bw[:].rearrange("p (c j) -> p c j", j=8),
                bb_ps[:].rearrange("p (c j) -> p c j", j=8),
                ev.to_broadcast((128, 8, 8)), Alu.mult)
            for c in range(8):
                nc.tensor.matmul(ANGM[:, 0:5],
                                 RT[:, c * 128:(c + 1) * 128],
                                 bw[:, c * 8:c * 8 + 5],
                                 start=(c == 0), stop=(c == 7))
            for c in range(8):
                nc.tensor.matmul(ANGM[:, 5:8],
                                 RT[:, 1024 + c * 128:1024 + (c + 1) * 128],
                                 bw[:, c * 8 + 5:c * 8 + 8],
                                 start=(c == 0), stop=(c == 7))

            # ============ group B trig: the 8 M-quads ============
            s1L = sb.tile([128, 32], f32, tag="s1L")
            t1L = sb.tile([128, 32], f32, tag="t1L")
            wnL = sb.tile([128, 32], f32, tag="wnL")
            v2L = sb.tile([128, 32], f32, tag="v2L")

            nc.vector.scalar_tensor_tensor(
                s1L[:].rearrange("p (q d) -> p q d", d=4),
                ANGM[:].unsqueeze(2).to_broadcast((128, 8, 4)), K4,
                frepB[:].rearrange("p (q d) -> p q d", d=4),
                Alu.mult, Alu.add)
            nc.vector.tensor_scalar(t1L[:], s1L[:], MAGIC, None, Alu.add)
            nc.vector.scalar_tensor_tensor(
                wnL[:], t1L[:], MAGIC, s1L[:], Alu.subtract, Alu.subtract)
            nc.scalar.activation(v2L[:], wnL[:], Act.Sin, scale=-2.0 * PI)

            v2qv = v2L[:].rearrange("p (q f) -> p q f", f=4)

            # ---- chain B tiles (ops emitted last) ----
            m4 = sb.tile([128, 4], f32, tag="m4")
            m8 = sb.tile([128, 8], f32, tag="m8")
            edB = sb.tile([128, 16], f32, tag="edB")
            zb = sb.tile([128, 8], f32, tag="zb")
            zA = sb.tile([128, 4], f32, tag="zA")
            scr2 = sb.tile([128, 8], f32, tag="scr2")

            # ============ chain A spine (DVE) ============
            m01 = sb.tile([128, 8], f32, tag="m01")
            m5 = sb.tile([128, 16], f32, tag="m5")
            D16 = sb.tile([128, 16], f32, tag="D16")

            m01r = m01[:].rearrange("p (k a b) -> p k a b", a=2, b=2)
            nc.vector.tensor_tensor(
                m01r,
                v2qv[:, 0:4:3, 0:2].unsqueeze(2).to_broadcast((128, 2, 2, 2)),
                v2qv[:, 1:3, :].rearrange("p k (b a) -> p k a b", a=2, b=2),
                Alu.mult)

            def dve_mc(dst, L, H, da, db, bc, bt, S):
                v1, v0 = da >> (bc + 1), 1 << bc
                tbh = bt - S
                w1, w0 = db >> (tbh + 1), 1 << tbh
                ov = dst.rearrange(
                    "p (w1 tb w0 v1 cb v0) -> p w1 tb w0 v1 cb v0",
                    tb=2, cb=2, w0=w0, v0=v0, w1=w1, v1=v1)
                Lv = L.rearrange("p (v1 cb v0) -> p v1 cb v0", cb=2, v0=v0)
                Hv = H.rearrange("p (w1 tb w0) -> p w1 tb w0", tb=2, w0=w0)
                for cbit in range(2):
                    o_h = ov[:, :, :, :, :, cbit, :]
                    Lh = Lv[:, :, cbit, :].unsqueeze(1).unsqueeze(1).unsqueeze(1)
                    Hh = Hv if cbit == 0 else Hv[:, :, ::-1, :]
                    Hh = Hh.unsqueeze(4).unsqueeze(5)
                    nc.vector.tensor_tensor(
                        o_h.squeeze(),
                        Lh.to_broadcast((128, w1, 2, w0, v1, v0)).squeeze(),
                        Hh.to_broadcast((128, w1, 2, w0, v1, v0)).squeeze(),
                        Alu.mult)

            dve_mc(m5[:], m01[:, 0:4], m01[:, 4:8], 4, 4, 1, 3, 2)

            def dve_ry(v, D, b, c_ap, s_ap, F, dst=None):
                if dst is None:
                    dst = v
                nc.vector.tensor_scalar(D[:, 0:F], v, s_ap, None, Alu.mult)
                vv = v.rearrange("p (o t i) -> p o t i", t=2, i=1 << b)
                dv = dst.rearrange("p (o t i) -> p o t i", t=2, i=1 << b)
                Dv = D[:, 0:F].rearrange("p (o t i) -> p o t i", t=2,
                                         i=1 << b)
                nc.vector.scalar_tensor_tensor(
                    dv[:, :, 0], vv[:, :, 0], c_ap, Dv[:, :, 1],
                    Alu.mult, Alu.subtract)
                nc.vector.scalar_tensor_tensor(
                    dv[:, :, 1], vv[:, :, 1], c_ap, Dv[:, :, 0],
                    Alu.mult, Alu.add)

            dve_ry(m5[:], D16, 0, cs_c(CS25), cs_s(CS25), 16)
            dve_ry(m5[:], D16, 3, cs_c(CS19), cs_s(CS19), 16)

            # ============ m6/pn side (Pool) ============
            def pool_mc(dst, L, H, da, db, bc, bt, S):
                v1, v0 = da >> (bc + 1), 1 << bc
                tbh = bt - S
                w1, w0 = db >> (tbh + 1), 1 << tbh
                ov = dst.rearrange(
                    "p (w1 tb w0 v1 cb v0) -> p w1 tb w0 v1 cb v0",
                    tb=2, cb=2, w0=w0, v0=v0, w1=w1, v1=v1)
                Lv = L.rearrange("p (v1 cb v0) -> p v1 cb v0", cb=2, v0=v0)
                Hv = H.rearrange("p (w1 tb w0) -> p w1 tb w0", tb=2, w0=w0)
                for cbit in range(2):
                    o_h = ov[:, :, :, :, :, cbit, :]
                    Lh = Lv[:, :, cbit, :].unsqueeze(1).unsqueeze(1).unsqueeze(1)
                    Hh = Hv if cbit == 0 else Hv[:, :, ::-1, :]
                    Hh = Hh.unsqueeze(4).unsqueeze(5)
                    nc.gpsimd.tensor_tensor(
                        o_h.squeeze(),
                        Lh.to_broadcast((128, w1, 2, w0, v1, v0)).squeeze(),
                        Hh.to_broadcast((128, w1, 2, w0, v1, v0)).squeeze(),
                        Alu.mult)

            m32 = sb.tile([128, 8], f32, tag="m32")
            m6 = sb.tile([128, 16], f32, tag="m6")
            pn = sb.tile([128, 32], f32, tag="pn")
            edA = sb.tile([128, 64], f32, tag="edA")
            edN = sb.tile([128, 64], f32, tag="edN")

            m32r = m32[:].rearrange("p (k a b) -> p k a b", a=2, b=2)
            nc.gpsimd.tensor_tensor(
                m32r,
                v2qv[:, 7:3:-3, 0:2].unsqueeze(2).to_broadcast((128, 2, 2, 2)),
                v2qv[:, 6:4:-1, :].rearrange("p k (b a) -> p k a b", a=2, b=2),
                Alu.mult)
            pool_mc(m6[:], m32[:, 0:4], m32[:, 4:8], 4, 4, 1, 3, 2)

            def pool_ry_ed(v, ed, pair_ap, b, F, dst):
                edv = ed[:, 0:2 * F].rearrange("p (t f) -> p t f", t=2)
                nc.gpsimd.tensor_tensor(
                    edv, v.unsqueeze(1).to_broadcast((128, 2, F)),
                    pair_ap.unsqueeze(2).to_broadcast((128, 2, F)), Alu.mult)
                E = ed[:, 0:F]
                D = ed[:, F:2 * F]
                i = 1 << b
                vv = dst.rearrange("p (o t i) -> p o t i", t=2, i=i)
                Ev = E.rearrange("p (o t i) -> p o t i", t=2, i=i)
                Dv = D.rearrange("p (o t i) -> p o t i", t=2, i=i)
                nc.gpsimd.tensor_tensor(vv[:, :, 0], Ev[:, :, 0], Dv[:, :, 1],
                                        Alu.subtract)
                nc.gpsimd.tensor_tensor(vv[:, :, 1], Ev[:, :, 1], Dv[:, :, 0],
                                        Alu.add)

            pool_ry_ed(m6[:], edA, cs_pair(CSPA), 3, 16, dst=pn[:, 0:16])
            dve_ry(m6[:], edN, 3, cs_c(CSNA), cs_s(CSNA), 16,
                   dst=pn[:, 16:32])

            # ============ measurement sums (DVE tail) ============
            zacc = sb.tile([128, 8], f32, tag="zacc")
            sq5 = sb.tile([128, 16], f32, tag="sq5")
            dD = sb.tile([128, 8], f32, tag="dD")
            scr3 = sb.tile([128, 32], f32, tag="scr3")
            nc.vector.tensor_tensor(sq5[:], m5[:], m5[:], Alu.mult)
            nc.vector.tensor_tensor(dD[:, 0:4], sq5[:, 0:8:2], sq5[:, 1:8:2],
                                    Alu.subtract)
            nc.vector.tensor_tensor(dD[:, 4:8], sq5[:, 9:16:2],
                                    sq5[:, 8:16:2], Alu.subtract)
            nc.vector.tensor_reduce(
                zacc[:, 0:2],
                dD[:].rearrange("p (a b) -> p a b", b=4), Ax.X, Alu.add)
            nc.vector.tensor_reduce(
                zacc[:, 2:4],
                sq5[:].rearrange("p (a b) -> p a b", b=8), Ax.X, Alu.add)
            nc.vector.scalar_tensor_tensor(
                scr3[:, 0:8], pn[:, 0:8], 2.0, pn[:, 0:8], Alu.mult, Alu.mult,
                accum_out=zacc[:, 4:5])
            nc.vector.scalar_tensor_tensor(
                scr3[:, 8:16], pn[:, 16:24], 2.0, pn[:, 16:24], Alu.mult,
                Alu.mult, accum_out=zacc[:, 5:6])
            nc.vector.scalar_tensor_tensor(
                scr3[:, 16:24], pn[:, 0:8], -2.0, pn[:, 8:16], Alu.mult,
                Alu.mult, accum_out=zacc[:, 6:7])
            nc.vector.scalar_tensor_tensor(
                scr3[:, 24:32], pn[:, 16:24], -2.0, pn[:, 24:32], Alu.mult,
                Alu.mult, accum_out=zacc[:, 7:8])

            # ============ final assembly ============
            out_sb = sb.tile([128, 2], f32, tag="out")
            cM14 = vA[:, 4:5]

            # z5: pairing [Sm, -Tm, Sp, Tp] x [2A0p, 2A0n, -2Qp, -2Qn]
            G = sb.tile([128, 4], f32, tag="G")
            zf = sb.tile([128, 4], f32, tag="zf")
            nc.vector.tensor_tensor(
                G[:].rearrange("p (a b) -> p a b", b=2),
                zacc[:, 4:8].rearrange("p (a b) -> p a b", b=2),
                cs_pair(CS29).unsqueeze(2).to_broadcast((128, 2, 2)),
                Alu.mult)
            nc.vector.scalar_tensor_tensor(
                zf[:, 0:4], G[:], 1.0, zacc[:, 0:4], Alu.mult, Alu.mult,
                accum_out=zf[:, 0:1])
            nc.scalar.activation(zf[:, 1:2], zacc[:, 0:1], Act.Identity,
                                 scale=1.0, bias=zacc[:, 1:2])
            nc.scalar.activation(zf[:, 2:3], zf[:, 1:2], Act.Identity,
                                 scale=cs_c(CS29))
            nc.scalar.activation(zf[:, 3:4], zf[:, 2:3], Act.Identity,
                                 scale=-1.0, bias=zf[:, 0:1])
            nc.scalar.activation(out_sb[:, 0:1], zf[:, 3:4], Act.Identity,
                                 scale=-PI, bias=pit[:])

            # ============ chain B (q10/z10): lowest priority ====

            nc.gpsimd.tensor_tensor(m4[:, 0:2], vA[:, 0:2], vA[:, 6:10:3],
                                    Alu.mult)
            nc.gpsimd.tensor_tensor(m4[:, 2:4], vA[:, 0:2], vA[:, 7:9],
                                    Alu.mult)

            pool_mc(m8[:], vA[:, 2:4], m4[:], 2, 4, 0, 2, 1)

            edv8 = edB[:, 0:16].rearrange("p (t f) -> p t f", t=2)
            nc.vector.tensor_tensor(
                edv8, m8[:].unsqueeze(1).to_broadcast((128, 2, 8)),
                cs_pair(CSB).unsqueeze(2).to_broadcast((128, 2, 8)), Alu.mult)
            m8v = m8[:].rearrange("p (o t i) -> p o t i", t=2, i=4)
            E8 = edB[:, 0:8].rearrange("p (o t i) -> p o t i", t=2, i=4)
            D8 = edB[:, 8:16].rearrange("p (o t i) -> p o t i", t=2, i=4)
            nc.vector.tensor_tensor(m8v[:, :, 0], E8[:, :, 0], D8[:, :, 1],
                                    Alu.subtract)
            nc.vector.tensor_tensor(m8v[:, :, 1], E8[:, :, 1], D8[:, :, 0],
                                    Alu.add)

            # z10 affine coefficients (vA-gated):
            # A1 = -pi*c30*cM14, A2 = pi*s30, A3 = pi*(1 + c30*cM14)
            nc.vector.tensor_tensor(zA[:, 3:4], cM14, vA[:, 18:19], Alu.mult)
            nc.vector.scalar_tensor_tensor(
                zA[:, 2:3], zA[:, 3:4], PI, pit[:], Alu.mult, Alu.add)
            nc.vector.tensor_scalar(zA[:, 0:1], zA[:, 3:4], -PI, None,
                                    Alu.mult)
            nc.vector.tensor_scalar(zA[:, 1:2], vA[:, 19:20], PI, None,
                                    Alu.mult)
            nc.vector.scalar_tensor_tensor(
                scr2[:, 0:4], m8[:, 0:4], 2.0, m8[:, 4:8], Alu.mult, Alu.mult,
                accum_out=zb[:, 1:2])
            nc.vector.scalar_tensor_tensor(
                scr2[:, 4:8], m8[:, 0:4], 2.0, m8[:, 0:4], Alu.mult, Alu.mult,
                accum_out=zb[:, 0:1])
            # z10 late part: out1 = A1*zb0 + A2*zb1 + A3 (A's precomputed)
            nc.vector.scalar_tensor_tensor(
                zb[:, 6:7], zb[:, 0:1], zA[:, 0:1], zA[:, 2:3],
                Alu.mult, Alu.add)
            nc.vector.scalar_tensor_tensor(
                out_sb[:, 1:2], zb[:, 1:2], zA[:, 1:2], zb[:, 6:7],
                Alu.mult, Alu.add)

            nc.sync.dma_start(out_d, out_sb[:])


    nc.compile()
    return nc


def get_nc():
    if "nc" not in _cache:
        _cache["nc"] = _build_program()
    return _cache["nc"]


def kernel(X, e, Ri, Ro, theta):
    import ml_dtypes
    from concourse.bass_utils import run_bass_kernel_spmd

    nc = get_nc()
    X = np.asarray(X, dtype=np.float32)
    e = np.asarray(e, dtype=np.float32)
    theta = np.asarray(theta, dtype=np.float32)
    pack = np.zeros((128, PW), dtype=np.float32)
    pack[:, PX:PX + 5] = X
    pack[:, PE0:PE0 + 8] = e.reshape(8, 128).T
    pack[0:36, PCK:PCK + NCK] = build_ck58()
    pack[0:31, PTHR:PTHR + 128] = np.broadcast_to(theta[:, None], (31, 128))
    pack[31, PTHR:PTHR + 128] = 1.0
    pack[32:36, PTHR:PTHR + 128] = X[:, [1, 3, 4, 0]].T
    f8 = ml_dtypes.float8_e4m3fn

    def chunkT(m):
        return m.T.reshape(8, 128, 128).transpose(1, 0, 2).reshape(128, 1024)

    ri = np.asarray(Ri, dtype=np.float32)
    ro = np.asarray(Ro, dtype=np.float32)
    riro = np.concatenate([ri, ro], axis=1).astype(f8)
    rt = np.concatenate([chunkT(ri), chunkT(ro)], axis=1).astype(f8)
    in_map = {
        "RiRo_f8": np.ascontiguousarray(riro),
        "RT_f8": np.ascontiguousarray(rt),
        "pack": pack,
    }
    res = run_bass_kernel_spmd(
        nc, [dict(in_map) for _ in range(N_CORES)],
        core_ids=list(range(N_CORES)),
    )
    return res.results[0]["out"]
